# revision 1
# baseline (speedup 1.0000x reference)
"""LSTM cell kernel for Trainium2, SPMD over 8 NeuronCores.

Problem: nn_LstmCell — B=8192, D_IN=D_H=2048.
    g = x @ Wx.T + bx + h @ Wh.T + bh          # [B, 3H]
    gi, gm, go = split(g, 3)
    c_new = sigmoid(gm)*c + sigmoid(gi)*tanh(gm)
    h_new = sigmoid(go)*tanh(c_new)

Strategy:
  - Data-parallel over batch: each core owns 1024 rows of x/h/c.
  - Single fused GEMM: A = [x ‖ h] (K=4096), W = [Wx ‖ Wh] ([6144, 4096]).
    Computed transposed (gates on PSUM partitions, batch on free dim) so the
    per-gate bias folds into the ScalarE activation (per-partition bias) and
    sigmoid/tanh read PSUM directly.
  - bf16 matmul inputs (fp32 PSUM accumulation); elementwise math in fp32.
  - Weights streamed from HBM (one pass), activations resident in SBUF.

Host-side: layout transforms + bf16 casts (not counted in HW exec time).
"""

import os

import numpy as np
import ml_dtypes

N_CORES = 8
B = 8192
DH = 2048            # latent dim (= D_IN = D_H)
H3 = 3 * DH          # 6144 gate rows
K = 2 * DH           # 4096 contraction dim
BLOC = B // N_CORES  # 1024 batch rows per core
P = 128
KT = K // P          # 32 k-tiles
MT = H3 // P         # 48 gate-row tiles
DTL = DH // P        # 16 d-tiles per gate
NF = 512             # matmul free dim (one PSUM bank of fp32)
NH = BLOC // NF      # 2 batch halves

_BF16 = ml_dtypes.bfloat16

_CACHE = {}
LAST_RESULT = None  # BassKernelResults from the most recent run (for test.py)


def _split_multiwaits(nc):
    """This container's walrus build rejects >1 sync-wait on an engine
    instruction ("Too many sync wait commands"). Split extra waits into
    standalone EventSemaphore instructions on the same engine immediately
    before the instruction (same stall semantics: engines are in-order)."""
    import concourse.mybir as mybir

    f = nc.m.functions[0]
    for blk in f.blocks:
        new_insts = []
        for inst in blk.instructions:
            si = getattr(inst, "sync_info", None)
            ow = list(si.on_wait) if (si is not None and si.on_wait) else []
            if len(ow) > 1:
                for w in ow[:-1]:
                    new_insts.append(
                        mybir.InstEventSemaphore(
                            name=nc.get_next_instruction_name(),
                            engine=inst.engine,
                            ins=[],
                            outs=[],
                            sync_info=mybir.SyncInfo(on_wait=[w], on_update=[]),
                        )
                    )
                inst.sync_info = mybir.SyncInfo(
                    on_wait=[ow[-1]], on_update=list(si.on_update)
                )
            new_insts.append(inst)
        blk.instructions[:] = new_insts


def _build_bass(dtl=DTL):
    import concourse.bass as bass
    import concourse.mybir as mybir
    import concourse.tile as tile

    f32 = mybir.dt.float32
    bf16 = mybir.dt.bfloat16
    AF = mybir.ActivationFunctionType

    nc = bass.Bass("TRN2", name="lstm_cell")

    WH = nc.dram_tensor("WH", [MT, P, KT, P], bf16, kind="ExternalInput")
    AH = nc.dram_tensor("AH", [P, KT, BLOC], bf16, kind="ExternalInput")
    CT = nc.dram_tensor("CT", [DH, BLOC], f32, kind="ExternalInput")
    BIAS = nc.dram_tensor("BIAS", [P, MT], f32, kind="ExternalInput")
    HT = nc.dram_tensor("HT", [DH, BLOC], f32, kind="ExternalOutput")
    CNT = nc.dram_tensor("CNT", [DH, BLOC], f32, kind="ExternalOutput")

    with tile.TileContext(nc) as tc:
        with (
            tc.tile_pool(name="const", bufs=1) as const_pool,
            tc.tile_pool(name="wpool", bufs=2) as wpool,
            tc.tile_pool(name="cpool", bufs=2) as cpool,
            tc.tile_pool(name="epool", bufs=3) as epool,
            tc.tile_pool(name="psum", bufs=1, space="PSUM") as psum_pool,
        ):
            # Activations resident in SBUF; per-k-tile chunks so the first
            # d-tile's matmuls can start as soon as early k-tiles land.
            a_sb = const_pool.tile([P, KT, BLOC], bf16, name="a_sb")
            for kg in range(KT):
                # gpsimd queue: cheap issue, and keeps the A load off the SP
                # queue that streams the weight strips.
                nc.gpsimd.dma_start(
                    a_sb[:, kg : kg + 1, :],
                    AH[:, kg : kg + 1, :],
                )
            bias_sb = const_pool.tile([P, MT], f32, name="bias_sb")
            nc.sync.dma_start(bias_sb[:], BIAS[:])

            for d in range(dtl):
                # Stream this d-tile's three gate weight strips (1 MB each).
                strips = []
                for gi, g in enumerate("imo"):
                    mt = gi * DTL + d
                    w_sb = wpool.tile([P, KT, P], bf16, name=f"w_{g}", tag=f"w_{g}")
                    nc.sync.dma_start(w_sb[:], WH[mt])
                    strips.append(w_sb)

                c_tiles = []
                for nh in range(NH):
                    c_t = cpool.tile([P, NF], f32, name=f"c_{nh}", tag=f"c_{nh}")
                    nc.sync.dma_start(
                        c_t[:], CT[d * P : (d + 1) * P, nh * NF : (nh + 1) * NF]
                    )
                    c_tiles.append(c_t)

                # GEMM: 3 gates x 32 k-tiles x 2 batch halves.
                # One weight load feeds both batch halves.
                psums = {}
                for gi, g in enumerate("imo"):
                    for nh in range(NH):
                        psums[(g, nh)] = psum_pool.tile(
                            [P, NF], f32, name=f"ps_{g}{nh}", tag=f"ps_{g}{nh}"
                        )
                if d == 0:
                    # k-major: PE streams right behind the A-chunk DMAs
                    # instead of stalling on the full A load.
                    for kt in range(KT):
                        for gi, g in enumerate("imo"):
                            for nh in range(NH):
                                nc.tensor.matmul(
                                    psums[(g, nh)][:],
                                    strips[gi][:, kt, :],
                                    a_sb[:, kt, nh * NF : (nh + 1) * NF],
                                    start=(kt == 0),
                                    stop=(kt == KT - 1),
                                )
                else:
                    # gate-major: each gate's PSUM bank drains (ACT) while
                    # the next gate's matmuls run.
                    for gi, g in enumerate("imo"):
                        w_sb = strips[gi]
                        for kt in range(KT):
                            for nh in range(NH):
                                nc.tensor.matmul(
                                    psums[(g, nh)][:],
                                    w_sb[:, kt, :],
                                    a_sb[:, kt, nh * NF : (nh + 1) * NF],
                                    start=(kt == 0),
                                    stop=(kt == KT - 1),
                                )

                # Epilogue: gates + cell update, fp32.
                for nh in range(NH):
                    b_i = bias_sb[:, d : d + 1]
                    b_m = bias_sb[:, DTL + d : DTL + d + 1]
                    b_o = bias_sb[:, 2 * DTL + d : 2 * DTL + d + 1]

                    s_i = epool.tile([P, NF], f32, name="s_i", tag="s_i")
                    t_m = epool.tile([P, NF], f32, name="t_m", tag="t_m")
                    s_m = epool.tile([P, NF], f32, name="s_m", tag="s_m")
                    s_o = epool.tile([P, NF], f32, name="s_o", tag="s_o")
                    part = epool.tile([P, NF], f32, name="part", tag="part")
                    fc = epool.tile([P, NF], f32, name="fc", tag="fc")
                    c_new = epool.tile([P, NF], f32, name="c_new", tag="c_new")
                    t_c = epool.tile([P, NF], f32, name="t_c", tag="t_c")
                    h_new = epool.tile([P, NF], f32, name="h_new", tag="h_new")

                    nc.scalar.activation(s_i[:], psums[("i", nh)][:], AF.Sigmoid, bias=b_i)
                    nc.scalar.activation(t_m[:], psums[("m", nh)][:], AF.Tanh, bias=b_m)
                    nc.scalar.activation(s_m[:], psums[("m", nh)][:], AF.Sigmoid, bias=b_m)
                    nc.scalar.activation(s_o[:], psums[("o", nh)][:], AF.Sigmoid, bias=b_o)
                    nc.vector.tensor_mul(part[:], s_i[:], t_m[:])
                    nc.vector.tensor_mul(fc[:], s_m[:], c_tiles[nh][:])
                    nc.vector.tensor_add(c_new[:], fc[:], part[:])
                    nc.scalar.activation(t_c[:], c_new[:], AF.Tanh)
                    nc.vector.tensor_mul(h_new[:], s_o[:], t_c[:])

                    nc.sync.dma_start(
                        CNT[d * P : (d + 1) * P, nh * NF : (nh + 1) * NF], c_new[:]
                    )
                    nc.sync.dma_start(
                        HT[d * P : (d + 1) * P, nh * NF : (nh + 1) * NF], h_new[:]
                    )

    _split_multiwaits(nc)
    return nc


def _get_bass():
    if "nc" not in _CACHE:
        _CACHE["nc"] = _build_bass()
    return _CACHE["nc"]


def _prepare_in_maps(x, h, c, Wix, bix, Wmx, bmx, Wox, box, Wih, bih, Wmh, bmh, Woh, boh):
    x = np.asarray(x, dtype=np.float32)
    h = np.asarray(h, dtype=np.float32)
    c = np.asarray(c, dtype=np.float32)

    # W = [Wx ‖ Wh] with gate rows [i, m, o]: [6144, 4096]
    W_full = np.concatenate(
        [
            np.concatenate([np.asarray(Wix), np.asarray(Wmx), np.asarray(Wox)], axis=0),
            np.concatenate([np.asarray(Wih), np.asarray(Wmh), np.asarray(Woh)], axis=0),
        ],
        axis=1,
    ).astype(np.float32)
    # WH[mt, p, kt, f] = W_full[mt*128+f, kt*128+p]
    WH_host = np.ascontiguousarray(
        W_full.reshape(MT, P, KT, P).transpose(0, 3, 2, 1)
    ).astype(_BF16)

    # A = [x ‖ h] : [8192, 4096] -> per-core [p, kt, n]
    A = np.concatenate([x, h], axis=1)
    AH_host = np.ascontiguousarray(
        A.reshape(N_CORES, BLOC, KT, P).transpose(0, 3, 2, 1)
    ).astype(_BF16)

    # c transposed per core: [core, 2048, 1024]
    CT_host = np.ascontiguousarray(c.reshape(N_CORES, BLOC, DH).transpose(0, 2, 1))

    bias = np.concatenate(
        [
            np.asarray(bix) + np.asarray(bih),
            np.asarray(bmx) + np.asarray(bmh),
            np.asarray(box) + np.asarray(boh),
        ]
    ).astype(np.float32)
    BIAS_host = np.ascontiguousarray(bias.reshape(MT, P).T)

    return [
        {
            "WH": WH_host,
            "AH": AH_host[core],
            "CT": CT_host[core],
            "BIAS": BIAS_host,
        }
        for core in range(N_CORES)
    ]


def _postprocess(results):
    """results: per-core list of {'HT': [2048,1024], 'CNT': [2048,1024]}."""
    h_new = (
        np.stack([np.asarray(results[core]["HT"]) for core in range(N_CORES)])
        .transpose(0, 2, 1)
        .reshape(B, DH)
        .astype(np.float32)
    )
    c_new = (
        np.stack([np.asarray(results[core]["CNT"]) for core in range(N_CORES)])
        .transpose(0, 2, 1)
        .reshape(B, DH)
        .astype(np.float32)
    )
    return (h_new, c_new)


def kernel(x, h, c, Wix, bix, Wmx, bmx, Wox, box, Wih, bih, Wmh, bmh, Woh, boh):
    global LAST_RESULT
    from concourse.bass_utils import run_bass_kernel_spmd

    in_maps = _prepare_in_maps(
        x, h, c, Wix, bix, Wmx, bmx, Wox, box, Wih, bih, Wmh, bmh, Woh, boh
    )
    nc = _get_bass()
    try:
        res = run_bass_kernel_spmd(nc, in_maps, core_ids=list(range(N_CORES)))
    except ModuleNotFoundError:
        # BASS_TRACE under axon needs antenv.axon_hooks, which some
        # containers lack; fall back to an untraced run.
        os.environ["BASS_NEVER_TRACE"] = "1"
        res = run_bass_kernel_spmd(nc, in_maps, core_ids=list(range(N_CORES)))
    LAST_RESULT = res
    return _postprocess(res.results)



# revision 9
# speedup vs baseline: 1.3864x; 1.3864x over previous
"""LSTM cell kernel for Trainium2, SPMD over 8 NeuronCores.

Problem: nn_LstmCell — B=8192, D_IN=D_H=2048.
    g = x @ Wx.T + bx + h @ Wh.T + bh          # [B, 3H]
    gi, gm, go = split(g, 3)
    c_new = sigmoid(gm)*c + sigmoid(gi)*tanh(gm)
    h_new = sigmoid(go)*tanh(c_new)

Strategy:
  - Data-parallel over batch: each core owns 1024 rows of x/h/c.
  - Fused GEMM computed transposed (gates on PSUM partitions, batch on the
    free dim) so per-gate biases fold into the ScalarE activation.
  - Mixed precision split by gate sensitivity: the m-gate pre-activation
    feeds tanh (derivative ~1) so it runs in bf16; the i/o gates only feed
    sigmoid (derivative <= 0.25) so they run in fp8 e4m3 with DoubleRow
    perf mode (K=256 per matmul, 2x PE throughput). Measured end-to-end
    rel err ~1.4e-2 vs 2.6e-2 for all-fp8 (gate: 2e-2).
  - fp8 scaling: A*32, W*512 (W's absmax 0.022 is below e4m3's min normal
    0.0156, so unscaled W would quantize to subnormals). The 1/16384
    descale folds into the sigmoid activation's scale operand.
  - DoubleRow matmuls may only write PSUM partitions 0..63, so each i/o
    gate accumulates its two 64-row halves into separate [64, 512] banks;
    the sigmoid ACTs then write the two partition halves of one [128, 512]
    SBUF tile (ScalarE supports input/output partition-base offsets).
  - Weights streamed from HBM (one pass), activations resident in SBUF.

Host-side: layout transforms + bf16/fp8 casts (not counted in HW exec time).
"""

import os

import numpy as np
import ml_dtypes

N_CORES = 8
B = 8192
DH = 2048            # latent dim (= D_IN = D_H)
K = 2 * DH           # 4096 contraction dim
BLOC = B // N_CORES  # 1024 batch rows per core
P = 128
KT = K // P          # 32 k-tiles (128 each)
KG = KT // 2         # 16 doubled k-groups (256 each) for fp8 DoubleRow
DTL = DH // P        # 16 d-tiles per gate
NF = 512             # matmul free dim (one PSUM bank of fp32)
NH = BLOC // NF      # 2 batch halves (vtiles per d-tile)

SA = 32.0            # fp8 activation scale
SW = 512.0           # fp8 weight scale
IO_DESCALE = 1.0 / (SA * SW)

_BF16 = ml_dtypes.bfloat16
_F8 = ml_dtypes.float8_e4m3

_CACHE = {}
LAST_RESULT = None  # BassKernelResults from the most recent run (for test.py)


def _split_multiwaits(nc):
    """This container's walrus build rejects >1 sync-wait on an engine
    instruction ("Too many sync wait commands"). Split extra waits into
    standalone EventSemaphore instructions on the same engine immediately
    before the instruction (same stall semantics: engines are in-order)."""
    import concourse.mybir as mybir

    f = nc.m.functions[0]
    for blk in f.blocks:
        new_insts = []
        for inst in blk.instructions:
            si = getattr(inst, "sync_info", None)
            ow = list(si.on_wait) if (si is not None and si.on_wait) else []
            if len(ow) > 1:
                for w in ow[:-1]:
                    new_insts.append(
                        mybir.InstEventSemaphore(
                            name=nc.get_next_instruction_name(),
                            engine=inst.engine,
                            ins=[],
                            outs=[],
                            sync_info=mybir.SyncInfo(on_wait=[w], on_update=[]),
                        )
                    )
                inst.sync_info = mybir.SyncInfo(
                    on_wait=[ow[-1]], on_update=list(si.on_update)
                )
            new_insts.append(inst)
        blk.instructions[:] = new_insts


def _build_bass(dtl=DTL):
    import concourse.bass as bass
    import concourse.mybir as mybir
    import concourse.tile as tile

    f32 = mybir.dt.float32
    bf16 = mybir.dt.bfloat16
    f8 = mybir.dt.float8e4
    AF = mybir.ActivationFunctionType
    DR = mybir.MatmulPerfMode.DoubleRow

    nc = bass.Bass("TRN2", name="lstm_cell")

    # m-gate weights, bf16: WM[d, p, kt, m] = Wm[d*128+m, kt*128+p]
    WM = nc.dram_tensor("WM", [dtl, P, KT, P], bf16, kind="ExternalInput")
    # i/o gate weights, fp8 DoubleRow layout:
    # W8[g*16+d, p, kg, ii, m] = Wg[d*128+m, kg*256+ii*128+p] * SW
    W8 = nc.dram_tensor("W8", [2 * dtl, P, KG, 2, P], f8, kind="ExternalInput")
    # activations A = [x ‖ h], twice: bf16 for m-gate, fp8*SA for i/o
    ABF = nc.dram_tensor("ABF", [P, KT, BLOC], bf16, kind="ExternalInput")
    A8 = nc.dram_tensor("A8", [P, KT, BLOC], f8, kind="ExternalInput")
    CT = nc.dram_tensor("CT", [DH, BLOC], f32, kind="ExternalInput")
    BIAS = nc.dram_tensor("BIAS", [P, 3 * dtl], f32, kind="ExternalInput")
    # i/o biases regrouped per 64-row half (DoubleRow outputs sit at
    # partitions 0..63): BIO[p, ((g*16+d)*2)+b] = bias_g[d*128+b*64+p]
    BIO = nc.dram_tensor("BIO", [64, 4 * dtl], f32, kind="ExternalInput")
    HT = nc.dram_tensor("HT", [DH, BLOC], f32, kind="ExternalOutput")
    CNT = nc.dram_tensor("CNT", [DH, BLOC], f32, kind="ExternalOutput")

    with tile.TileContext(nc) as tc:
        with (
            tc.tile_pool(name="const", bufs=1) as const_pool,
            tc.tile_pool(name="wpool", bufs=2) as wpool,
            tc.tile_pool(name="cpool", bufs=2) as cpool,
            tc.tile_pool(name="epool", bufs=3) as epool,
            tc.tile_pool(name="psum", bufs=1, space="PSUM") as psum_pool,
        ):
            # Activations resident in SBUF; per-k-chunk loads so the first
            # d-tile's matmuls can start as soon as early chunks land.
            # Both A streams go on the gpsimd queue, fp8 strictly first:
            # the model serializes all transfers through one shared DMA
            # resource, so only queue order keeps the big bf16 chunks (m-gate,
            # consumed last) from starving the fp8 chunks (i/o, consumed
            # first). Chunks sized to amortize the ~1us SWDGE prep per DMA.
            a8_sb = const_pool.tile([P, KT, BLOC], f8, name="a8_sb")
            for ch in range(8):
                nc.gpsimd.dma_start(
                    a8_sb[:, 4 * ch : 4 * ch + 4, :],
                    A8[:, 4 * ch : 4 * ch + 4, :],
                )
            # ABF chunk DMAs are emitted inside the d==0 body, after d0's
            # wm/c loads, so those beat the bulk bf16 stream to the DMA
            # engines (the m-gate is scheduled last within d0 anyway).
            abf_sb = const_pool.tile([P, KT, BLOC], bf16, name="abf_sb")
            bias_sb = const_pool.tile([P, 3 * dtl], f32, name="bias_sb")
            nc.sync.dma_start(bias_sb[:], BIAS[:])
            bio_sb = const_pool.tile([64, 4 * dtl], f32, name="bio_sb")
            nc.sync.dma_start(bio_sb[:], BIO[:])

            for d in range(dtl):
                # Stream this d-tile's weight strips: fp8 i/o (0.5 MB each)
                # + bf16 m (1 MB).
                w8 = {}
                for gi, g in enumerate("io"):
                    w8[g] = wpool.tile([P, KG, 2, P], f8, name=f"w8{g}", tag=f"w8{g}")
                    nc.sync.dma_start(w8[g][:], W8[gi * dtl + d])
                wm = wpool.tile([P, KT, P], bf16, name="wm", tag="wm")
                nc.sync.dma_start(wm[:], WM[d])

                c_tiles, psums = {}, {}
                for nh in range(NH):
                    c_t = cpool.tile([P, NF], f32, name=f"c_{nh}", tag=f"c_{nh}")
                    nc.sync.dma_start(
                        c_t[:], CT[d * P : (d + 1) * P, nh * NF : (nh + 1) * NF]
                    )
                    c_tiles[nh] = c_t
                    # m-gate: one full bank; parity tags so consecutive
                    # vtiles overlap. i/o: [64, 512] banks (DoubleRow dst
                    # partition must be 0), one per 64-row half.
                    psums[("m", nh)] = psum_pool.tile(
                        [P, NF], f32, name=f"ps_m{nh}", tag=f"ps_m{nh}"
                    )
                    for g in "io":
                        for b in range(2):
                            psums[(g, nh, b)] = psum_pool.tile(
                                [64, NF], f32, name=f"ps_{g}{b}", tag=f"ps_{g}{b}"
                            )

                def io_matmul(g, nh, b, kg):
                    # fp8 DoubleRow: K=256 (k-tile pair), M=64, N=512.
                    nc.tensor.matmul(
                        psums[(g, nh, b)][:],
                        w8[g][:, kg, :, b * 64 : (b + 1) * 64],
                        a8_sb[:, 2 * kg : 2 * kg + 2, nh * NF : (nh + 1) * NF],
                        start=(kg == 0),
                        stop=(kg == KG - 1),
                        perf_mode=DR,
                    )

                def m_matmul(nh, kt):
                    nc.tensor.matmul(
                        psums[("m", nh)][:],
                        wm[:, kt, :],
                        abf_sb[:, kt, nh * NF : (nh + 1) * NF],
                        start=(kt == 0),
                        stop=(kt == KT - 1),
                    )

                def epilogue(nh):
                    b_m = bias_sb[:, dtl + d : dtl + d + 1]

                    s_i = epool.tile([P, NF], f32, name="s_i", tag="s_i")
                    t_m = epool.tile([P, NF], f32, name="t_m", tag="t_m")
                    s_m = epool.tile([P, NF], f32, name="s_m", tag="s_m")
                    s_o = epool.tile([P, NF], f32, name="s_o", tag="s_o")
                    part = epool.tile([P, NF], f32, name="part", tag="part")
                    fc = epool.tile([P, NF], f32, name="fc", tag="fc")
                    c_new = epool.tile([P, NF], f32, name="c_new", tag="c_new")
                    t_c = epool.tile([P, NF], f32, name="t_c", tag="t_c")
                    h_new = epool.tile([P, NF], f32, name="h_new", tag="h_new")

                    # i/o halves: PSUM [64, 512] at partition base 0 ->
                    # partition halves of the [128, 512] SBUF tile.
                    for gi, (g, s_g) in enumerate((("i", s_i), ("o", s_o))):
                        for b in range(2):
                            nc.scalar.activation(
                                s_g[b * 64 : (b + 1) * 64, :],
                                psums[(g, nh, b)][:],
                                AF.Sigmoid,
                                bias=bio_sb[
                                    :, (gi * dtl + d) * 2 + b : (gi * dtl + d) * 2 + b + 1
                                ],
                                scale=IO_DESCALE,
                            )
                    nc.scalar.activation(t_m[:], psums[("m", nh)][:], AF.Tanh, bias=b_m)
                    nc.scalar.activation(s_m[:], psums[("m", nh)][:], AF.Sigmoid, bias=b_m)
                    nc.vector.tensor_mul(part[:], s_i[:], t_m[:])
                    nc.vector.tensor_mul(fc[:], s_m[:], c_tiles[nh][:])
                    nc.vector.tensor_add(c_new[:], fc[:], part[:])
                    nc.scalar.activation(t_c[:], c_new[:], AF.Tanh)
                    nc.vector.tensor_mul(h_new[:], s_o[:], t_c[:])

                    nc.gpsimd.dma_start(
                        CNT[d * P : (d + 1) * P, nh * NF : (nh + 1) * NF], c_new[:]
                    )
                    nc.gpsimd.dma_start(
                        HT[d * P : (d + 1) * P, nh * NF : (nh + 1) * NF], h_new[:]
                    )

                if d == 0:
                    # k-major: PE streams right behind the A-chunk DMAs
                    # instead of stalling on the full A loads. All i/o work
                    # (fp8 A, lands first) before any m work (bf16 A, lands
                    # second); m matmuls interleave both vtiles k-major so
                    # they chase the ABF chunk DMAs without a serial tail.
                    for nh in range(NH):
                        for g in "io":
                            for kg in range(KG):
                                for b in range(2):
                                    io_matmul(g, nh, b, kg)
                    for kt in range(KT):
                        for nh in range(NH):
                            m_matmul(nh, kt)
                    for nh in range(NH):
                        epilogue(nh)
                else:
                    # gate-major per vtile: each gate's PSUM drains (ACT)
                    # while the next gate's matmuls run.
                    for nh in range(NH):
                        for b in range(2):
                            for kg in range(KG):
                                io_matmul("i", nh, b, kg)
                        for kt in range(KT):
                            m_matmul(nh, kt)
                        for b in range(2):
                            for kg in range(KG):
                                io_matmul("o", nh, b, kg)
                        epilogue(nh)

    _split_multiwaits(nc)
    return nc


def _get_bass():
    if "nc" not in _CACHE:
        _CACHE["nc"] = _build_bass()
    return _CACHE["nc"]


def _prepare_in_maps(x, h, c, Wix, bix, Wmx, bmx, Wox, box, Wih, bih, Wmh, bmh, Woh, boh):
    x = np.asarray(x, dtype=np.float32)
    h = np.asarray(h, dtype=np.float32)
    c = np.asarray(c, dtype=np.float32)

    # Per-gate fused weights [2048, 4096]: W = [Wx ‖ Wh]
    Wg = {
        "i": np.concatenate([np.asarray(Wix), np.asarray(Wih)], axis=1),
        "m": np.concatenate([np.asarray(Wmx), np.asarray(Wmh)], axis=1),
        "o": np.concatenate([np.asarray(Wox), np.asarray(Woh)], axis=1),
    }

    # m-gate bf16: WM[d, p, kt, m] = Wm[d*128+m, kt*128+p]
    WM_host = np.ascontiguousarray(
        Wg["m"].astype(np.float32).reshape(DTL, P, KT, P).transpose(0, 3, 2, 1)
    ).astype(_BF16)

    # i/o gates fp8 (scaled by SW), DoubleRow layout:
    # W8[g*16+d, p, kg, ii, m] = Wg[d*128+m, kg*256+ii*128+p]*SW
    w8_list = []
    for g in "io":
        ws = (Wg[g].astype(np.float32) * SW).astype(_F8)
        w8_list.append(ws.reshape(DTL, P, KG, 2, P).transpose(0, 4, 2, 3, 1))
    W8_host = np.ascontiguousarray(np.concatenate(w8_list, axis=0))

    # A = [x ‖ h] : [8192, 4096] -> per-core [p, kt, n], in bf16 and fp8*SA
    A = np.concatenate([x, h], axis=1)
    A_t = A.reshape(N_CORES, BLOC, KT, P).transpose(0, 3, 2, 1)
    ABF_host = np.ascontiguousarray(A_t).astype(_BF16)
    A8_host = np.ascontiguousarray(A_t * np.float32(SA)).astype(_F8)

    # c transposed per core: [core, 2048, 1024]
    CT_host = np.ascontiguousarray(c.reshape(N_CORES, BLOC, DH).transpose(0, 2, 1))

    bias = {g: (np.asarray(bx) + np.asarray(bh)).astype(np.float32)
            for g, bx, bh in (("i", bix, bih), ("m", bmx, bmh), ("o", box, boh))}
    BIAS_host = np.ascontiguousarray(
        np.concatenate([bias["i"], bias["m"], bias["o"]]).reshape(3 * DTL, P).T
    )
    # BIO[p, (g*16+d)*2+b] = bias_g[d*128+b*64+p] for g in (i, o)
    BIO_host = np.ascontiguousarray(
        np.concatenate([bias["i"], bias["o"]]).reshape(4 * DTL, 64).T
    )

    return [
        {
            "WM": WM_host,
            "W8": W8_host,
            "ABF": ABF_host[core],
            "A8": A8_host[core],
            "CT": CT_host[core],
            "BIAS": BIAS_host,
            "BIO": BIO_host,
        }
        for core in range(N_CORES)
    ]


def _postprocess(results):
    """results: per-core list of {'HT': [2048,1024], 'CNT': [2048,1024]}."""
    h_new = (
        np.stack([np.asarray(results[core]["HT"]) for core in range(N_CORES)])
        .transpose(0, 2, 1)
        .reshape(B, DH)
        .astype(np.float32)
    )
    c_new = (
        np.stack([np.asarray(results[core]["CNT"]) for core in range(N_CORES)])
        .transpose(0, 2, 1)
        .reshape(B, DH)
        .astype(np.float32)
    )
    return (h_new, c_new)


def kernel(x, h, c, Wix, bix, Wmx, bmx, Wox, box, Wih, bih, Wmh, bmh, Woh, boh):
    global LAST_RESULT
    from concourse.bass_utils import run_bass_kernel_spmd

    in_maps = _prepare_in_maps(
        x, h, c, Wix, bix, Wmx, bmx, Wox, box, Wih, bih, Wmh, bmh, Woh, boh
    )
    nc = _get_bass()
    try:
        res = run_bass_kernel_spmd(nc, in_maps, core_ids=list(range(N_CORES)))
    except ModuleNotFoundError:
        # BASS_TRACE under axon needs antenv.axon_hooks, which some
        # containers lack; fall back to an untraced run.
        os.environ["BASS_NEVER_TRACE"] = "1"
        res = run_bass_kernel_spmd(nc, in_maps, core_ids=list(range(N_CORES)))
    LAST_RESULT = res
    return _postprocess(res.results)


# revision 21
# speedup vs baseline: 1.4648x; 1.0566x over previous
"""LSTM cell kernel for Trainium2, SPMD over 8 NeuronCores.

Problem: nn_LstmCell — B=8192, D_IN=D_H=2048.
    g = x @ Wx.T + bx + h @ Wh.T + bh          # [B, 3H]
    gi, gm, go = split(g, 3)
    c_new = sigmoid(gm)*c + sigmoid(gi)*tanh(gm)
    h_new = sigmoid(go)*tanh(c_new)

Strategy:
  - Data-parallel over batch: each core owns 1024 rows of x/h/c.
  - Fused GEMM computed transposed (gates on PSUM partitions, batch on the
    free dim) so per-gate biases fold into the ScalarE activation.
  - Mixed precision split by gate sensitivity: the m-gate pre-activation
    feeds tanh (derivative ~1) so it runs in bf16; the i/o gates only feed
    sigmoid (derivative <= 0.25) so they run in fp8 e4m3 with DoubleRow
    perf mode (K=256 per matmul, 2x PE throughput). Measured end-to-end
    rel err ~1.4e-2 vs 2.6e-2 for all-fp8 (gate: 2e-2).
  - fp8 scaling: A*32, W*512 (W's absmax 0.022 is below e4m3's min normal
    0.0156, so unscaled W would quantize to subnormals). The 1/16384
    descale folds into the sigmoid activation's scale operand.
  - DoubleRow matmuls may only write PSUM partitions 0..63, so each i/o
    gate accumulates its two 64-row halves into separate [64, 512] banks;
    the sigmoid ACTs then write the two partition halves of one [128, 512]
    SBUF tile (ScalarE supports input/output partition-base offsets).
  - Weights streamed from HBM (one pass), activations resident in SBUF.

Host-side: layout transforms + bf16/fp8 casts (not counted in HW exec time).
"""

import os

import numpy as np
import ml_dtypes

N_CORES = 8
B = 8192
DH = 2048            # latent dim (= D_IN = D_H)
K = 2 * DH           # 4096 contraction dim
BLOC = B // N_CORES  # 1024 batch rows per core
P = 128
KT = K // P          # 32 k-tiles (128 each)
KG = KT // 2         # 16 doubled k-groups (256 each) for fp8 DoubleRow
DTL = DH // P        # 16 d-tiles per gate
NF = 512             # matmul free dim (one PSUM bank of fp32)
NH = BLOC // NF      # 2 batch halves (vtiles per d-tile)

SA = 32.0            # fp8 activation scale
SW = 512.0           # fp8 weight scale
IO_DESCALE = 1.0 / (SA * SW)

_BF16 = ml_dtypes.bfloat16
_F8 = ml_dtypes.float8_e4m3

_CACHE = {}
LAST_RESULT = None  # BassKernelResults from the most recent run (for test.py)


def _split_multiwaits(nc):
    """This container's walrus build rejects >1 sync-wait on an engine
    instruction ("Too many sync wait commands"). Split extra waits into
    standalone EventSemaphore instructions on the same engine immediately
    before the instruction (same stall semantics: engines are in-order)."""
    import concourse.mybir as mybir

    f = nc.m.functions[0]
    for blk in f.blocks:
        new_insts = []
        for inst in blk.instructions:
            si = getattr(inst, "sync_info", None)
            ow = list(si.on_wait) if (si is not None and si.on_wait) else []
            if len(ow) > 1:
                for w in ow[:-1]:
                    new_insts.append(
                        mybir.InstEventSemaphore(
                            name=nc.get_next_instruction_name(),
                            engine=inst.engine,
                            ins=[],
                            outs=[],
                            sync_info=mybir.SyncInfo(on_wait=[w], on_update=[]),
                        )
                    )
                inst.sync_info = mybir.SyncInfo(
                    on_wait=[ow[-1]], on_update=list(si.on_update)
                )
            new_insts.append(inst)
        blk.instructions[:] = new_insts


def _build_bass(dtl=DTL):
    import concourse.bass as bass
    import concourse.mybir as mybir
    import concourse.tile as tile

    f32 = mybir.dt.float32
    bf16 = mybir.dt.bfloat16
    f8 = mybir.dt.float8e4
    AF = mybir.ActivationFunctionType
    DR = mybir.MatmulPerfMode.DoubleRow

    nc = bass.Bass("TRN2", name="lstm_cell")

    # m-gate weights, bf16: WM[d, p, kt, m] = Wm[d*128+m, kt*128+p]
    WM = nc.dram_tensor("WM", [dtl, P, KT, P], bf16, kind="ExternalInput")
    # i/o gate weights, fp8 DoubleRow layout, plus one extra strip (index
    # 2*dtl) holding the M-GATE's d=0 rows in fp8: d-tile 0 computes even
    # the m-gate in fp8 so it depends only on the (small, early) fp8 A
    # stream — the 8 MB bf16 A stream then loads during d1+ instead of
    # stalling the prologue. Costs ~1e-3 of rel err (1/16 of the m-gate).
    # W8[g*16+d, p, kg, ii, m] = Wg[d*128+m, kg*256+ii*128+p] * SW
    W8 = nc.dram_tensor("W8", [2 * dtl + 1, P, KG, 2, P], f8, kind="ExternalInput")
    # activations A = [x ‖ h], twice: bf16 for m-gate, fp8*SA for i/o
    ABF = nc.dram_tensor("ABF", [P, KT, BLOC], bf16, kind="ExternalInput")
    A8 = nc.dram_tensor("A8", [P, KT, BLOC], f8, kind="ExternalInput")
    CT = nc.dram_tensor("CT", [DH, BLOC], f32, kind="ExternalInput")
    BIAS = nc.dram_tensor("BIAS", [P, 3 * dtl], f32, kind="ExternalInput")
    # i/o biases regrouped per 64-row half (DoubleRow outputs sit at
    # partitions 0..63): BIO[p, ((g*16+d)*2)+b] = bias_g[d*128+b*64+p].
    # Cols 4*dtl + b hold the m-gate's d=0 bias halves (fp8 d-tile 0).
    BIO = nc.dram_tensor("BIO", [64, 4 * dtl + 2], f32, kind="ExternalInput")
    HT = nc.dram_tensor("HT", [DH, BLOC], f32, kind="ExternalOutput")
    CNT = nc.dram_tensor("CNT", [DH, BLOC], f32, kind="ExternalOutput")

    with tile.TileContext(nc) as tc:
        with (
            tc.tile_pool(name="const", bufs=1) as const_pool,
            tc.tile_pool(name="wpool", bufs=2) as wpool,
            tc.tile_pool(name="cpool", bufs=2) as cpool,
            tc.tile_pool(name="epool", bufs=3) as epool,
            tc.tile_pool(name="psum", bufs=1, space="PSUM") as psum_pool,
        ):
            # Activations resident in SBUF; per-k-chunk loads so the first
            # d-tile's matmuls can start as soon as early chunks land.
            # Both A streams go on the gpsimd queue, fp8 strictly first:
            # the model serializes all transfers through one shared DMA
            # resource, so only queue order keeps the big bf16 chunks (m-gate,
            # consumed last) from starving the fp8 chunks (i/o, consumed
            # first). Chunks sized to amortize the ~1us SWDGE prep per DMA.
            a8_sb = const_pool.tile([P, KT, BLOC], f8, name="a8_sb")
            for ch in range(8):
                nc.gpsimd.dma_start(
                    a8_sb[:, 4 * ch : 4 * ch + 4, :],
                    A8[:, 4 * ch : 4 * ch + 4, :],
                )
            # ABF chunk DMAs are emitted inside the d==0 body, after d0's
            # wm/c loads, so those beat the bulk bf16 stream to the DMA
            # engines (the m-gate is scheduled last within d0 anyway).
            abf_sb = const_pool.tile([P, KT, BLOC], bf16, name="abf_sb")
            bias_sb = const_pool.tile([P, 3 * dtl], f32, name="bias_sb")
            nc.sync.dma_start(bias_sb[:], BIAS[:])
            bio_sb = const_pool.tile([64, 4 * dtl + 2], f32, name="bio_sb")
            nc.sync.dma_start(bio_sb[:], BIO[:])

            for d in range(dtl):
                # Stream this d-tile's weight strips: fp8 i/o (0.5 MB each)
                # + bf16 m (1 MB).
                # d0's c rides the gpsimd queue between the A8 and ABF
                # streams: it would otherwise win the shared DMA engines
                # ahead of the latency-critical fp8 chunks.
                d0_eng = nc.gpsimd if d == 0 else nc.sync
                w8 = {}
                for gi, g in enumerate("io"):
                    w8[g] = wpool.tile([P, KG, 2, P], f8, name=f"w8{g}", tag=f"w8{g}")
                    nc.sync.dma_start(w8[g][:], W8[gi * dtl + d])
                if d == 0:
                    w8["m"] = wpool.tile([P, KG, 2, P], f8, name="w8m", tag="w8m")
                    nc.sync.dma_start(w8["m"][:], W8[2 * dtl])
                else:
                    wm = wpool.tile([P, KT, P], bf16, name="wm", tag="wm")
                    nc.sync.dma_start(wm[:], WM[d])

                c_tiles, psums = {}, {}
                for nh in range(NH):
                    c_t = cpool.tile([P, NF], f32, name=f"c_{nh}", tag=f"c_{nh}")
                    d0_eng.dma_start(
                        c_t[:], CT[d * P : (d + 1) * P, nh * NF : (nh + 1) * NF]
                    )
                    c_tiles[nh] = c_t
                    # m-gate: one full bank; parity tags so consecutive
                    # vtiles overlap. i/o: [64, 512] banks (DoubleRow dst
                    # partition must be 0), one per 64-row half. d0's fp8
                    # m-gate gets its own [64, 512] pair (8 banks total).
                    if d == 0:
                        for b in range(2):
                            psums[("m8", nh, b)] = psum_pool.tile(
                                [64, NF], f32, name=f"ps_m8{b}", tag=f"ps_m8{b}"
                            )
                    else:
                        psums[("m", nh)] = psum_pool.tile(
                            [P, NF], f32, name=f"ps_m{nh}", tag=f"ps_m{nh}"
                        )
                    for g in "io":
                        for b in range(2):
                            psums[(g, nh, b)] = psum_pool.tile(
                                [64, NF], f32, name=f"ps_{g}{b}", tag=f"ps_{g}{b}"
                            )

                def io_matmul(g, nh, b, kg, key=None):
                    # fp8 DoubleRow: K=256 (k-tile pair), M=64, N=512.
                    nc.tensor.matmul(
                        psums[(key or g, nh, b)][:],
                        w8[g][:, kg, :, b * 64 : (b + 1) * 64],
                        a8_sb[:, 2 * kg : 2 * kg + 2, nh * NF : (nh + 1) * NF],
                        start=(kg == 0),
                        stop=(kg == KG - 1),
                        perf_mode=DR,
                    )

                def m_matmul(nh, kt):
                    nc.tensor.matmul(
                        psums[("m", nh)][:],
                        wm[:, kt, :],
                        abf_sb[:, kt, nh * NF : (nh + 1) * NF],
                        start=(kt == 0),
                        stop=(kt == KT - 1),
                    )

                def epilogue(nh):
                    b_m = bias_sb[:, dtl + d : dtl + d + 1]

                    s_i = epool.tile([P, NF], f32, name="s_i", tag="s_i")
                    t_m = epool.tile([P, NF], f32, name="t_m", tag="t_m")
                    s_m = epool.tile([P, NF], f32, name="s_m", tag="s_m")
                    s_o = epool.tile([P, NF], f32, name="s_o", tag="s_o")
                    part = epool.tile([P, NF], f32, name="part", tag="part")
                    fc = epool.tile([P, NF], f32, name="fc", tag="fc")
                    c_new = epool.tile([P, NF], f32, name="c_new", tag="c_new")
                    t_c = epool.tile([P, NF], f32, name="t_c", tag="t_c")
                    h_new = epool.tile([P, NF], f32, name="h_new", tag="h_new")

                    # i/o halves: PSUM [64, 512] at partition base 0 ->
                    # partition halves of the [128, 512] SBUF tile.
                    for gi, (g, s_g) in enumerate((("i", s_i), ("o", s_o))):
                        for b in range(2):
                            nc.scalar.activation(
                                s_g[b * 64 : (b + 1) * 64, :],
                                psums[(g, nh, b)][:],
                                AF.Sigmoid,
                                bias=bio_sb[
                                    :, (gi * dtl + d) * 2 + b : (gi * dtl + d) * 2 + b + 1
                                ],
                                scale=IO_DESCALE,
                            )
                    if d == 0:
                        for b in range(2):
                            bm8 = bio_sb[:, 4 * dtl + b : 4 * dtl + b + 1]
                            nc.scalar.activation(
                                t_m[b * 64 : (b + 1) * 64, :],
                                psums[("m8", nh, b)][:],
                                AF.Tanh, bias=bm8, scale=IO_DESCALE,
                            )
                            nc.scalar.activation(
                                s_m[b * 64 : (b + 1) * 64, :],
                                psums[("m8", nh, b)][:],
                                AF.Sigmoid, bias=bm8, scale=IO_DESCALE,
                            )
                    else:
                        nc.scalar.activation(t_m[:], psums[("m", nh)][:], AF.Tanh, bias=b_m)
                        nc.scalar.activation(s_m[:], psums[("m", nh)][:], AF.Sigmoid, bias=b_m)
                    nc.vector.tensor_mul(part[:], s_i[:], t_m[:])
                    nc.vector.tensor_mul(fc[:], s_m[:], c_tiles[nh][:])
                    nc.vector.tensor_add(c_new[:], fc[:], part[:])
                    nc.scalar.activation(t_c[:], c_new[:], AF.Tanh)
                    nc.vector.tensor_mul(h_new[:], s_o[:], t_c[:])

                    nc.gpsimd.dma_start(
                        CNT[d * P : (d + 1) * P, nh * NF : (nh + 1) * NF], c_new[:]
                    )
                    nc.gpsimd.dma_start(
                        HT[d * P : (d + 1) * P, nh * NF : (nh + 1) * NF], h_new[:]
                    )

                if d == 0:
                    # Emit the bulk bf16 A stream (needed first by d1's
                    # m-gate) behind d0's c loads on the same queue.
                    for ch in range(8):
                        nc.gpsimd.dma_start(
                            abf_sb[:, 4 * ch : 4 * ch + 4, :],
                            ABF[:, 4 * ch : 4 * ch + 4, :],
                        )
                    # d0 is all-fp8 (m included): k-major so the PE streams
                    # right behind the fp8 A-chunk DMAs; no dependence on
                    # the bf16 stream at all.
                    for nh in range(NH):
                        for g in "iom":
                            for kg in range(KG):
                                for b in range(2):
                                    io_matmul(g, nh, b, kg, key="m8" if g == "m" else None)
                        epilogue(nh)
                else:
                    # gate-major per vtile, m last: s_i/s_o drain their PSUM
                    # banks during the m matmuls, shortening the per-vtile
                    # epilogue tail to the m-dependent chain.
                    for nh in range(NH):
                        for b in range(2):
                            for kg in range(KG):
                                io_matmul("i", nh, b, kg)
                        for b in range(2):
                            for kg in range(KG):
                                io_matmul("o", nh, b, kg)
                        for kt in range(KT):
                            m_matmul(nh, kt)
                        epilogue(nh)

    _split_multiwaits(nc)
    return nc


def _get_bass():
    if "nc" not in _CACHE:
        _CACHE["nc"] = _build_bass()
    return _CACHE["nc"]


def _prepare_in_maps(x, h, c, Wix, bix, Wmx, bmx, Wox, box, Wih, bih, Wmh, bmh, Woh, boh):
    x = np.asarray(x, dtype=np.float32)
    h = np.asarray(h, dtype=np.float32)
    c = np.asarray(c, dtype=np.float32)

    # Per-gate fused weights [2048, 4096]: W = [Wx ‖ Wh]
    Wg = {
        "i": np.concatenate([np.asarray(Wix), np.asarray(Wih)], axis=1),
        "m": np.concatenate([np.asarray(Wmx), np.asarray(Wmh)], axis=1),
        "o": np.concatenate([np.asarray(Wox), np.asarray(Woh)], axis=1),
    }

    # m-gate bf16: WM[d, p, kt, m] = Wm[d*128+m, kt*128+p]
    WM_host = np.ascontiguousarray(
        Wg["m"].astype(np.float32).reshape(DTL, P, KT, P).transpose(0, 3, 2, 1)
    ).astype(_BF16)

    # i/o gates fp8 (scaled by SW), DoubleRow layout:
    # W8[g*16+d, p, kg, ii, m] = Wg[d*128+m, kg*256+ii*128+p]*SW
    # plus the m-gate's d=0 strip at index 2*DTL (d-tile 0 runs all-fp8).
    w8_list = []
    for g in "io":
        ws = (Wg[g].astype(np.float32) * SW).astype(_F8)
        w8_list.append(ws.reshape(DTL, P, KG, 2, P).transpose(0, 4, 2, 3, 1))
    wm8 = (Wg["m"][:P].astype(np.float32) * SW).astype(_F8)
    w8_list.append(wm8.reshape(1, P, KG, 2, P).transpose(0, 4, 2, 3, 1))
    W8_host = np.ascontiguousarray(np.concatenate(w8_list, axis=0))

    # A = [x ‖ h] : [8192, 4096] -> per-core [p, kt, n], in bf16 and fp8*SA
    A = np.concatenate([x, h], axis=1)
    A_t = A.reshape(N_CORES, BLOC, KT, P).transpose(0, 3, 2, 1)
    ABF_host = np.ascontiguousarray(A_t).astype(_BF16)
    A8_host = np.ascontiguousarray(A_t * np.float32(SA)).astype(_F8)

    # c transposed per core: [core, 2048, 1024]
    CT_host = np.ascontiguousarray(c.reshape(N_CORES, BLOC, DH).transpose(0, 2, 1))

    bias = {g: (np.asarray(bx) + np.asarray(bh)).astype(np.float32)
            for g, bx, bh in (("i", bix, bih), ("m", bmx, bmh), ("o", box, boh))}
    BIAS_host = np.ascontiguousarray(
        np.concatenate([bias["i"], bias["m"], bias["o"]]).reshape(3 * DTL, P).T
    )
    # BIO[p, (g*16+d)*2+b] = bias_g[d*128+b*64+p] for g in (i, o);
    # trailing two cols: m-gate d=0 bias halves.
    BIO_host = np.ascontiguousarray(
        np.concatenate([bias["i"], bias["o"], bias["m"][:P]]).reshape(4 * DTL + 2, 64).T
    )

    return [
        {
            "WM": WM_host,
            "W8": W8_host,
            "ABF": ABF_host[core],
            "A8": A8_host[core],
            "CT": CT_host[core],
            "BIAS": BIAS_host,
            "BIO": BIO_host,
        }
        for core in range(N_CORES)
    ]


def _postprocess(results):
    """results: per-core list of {'HT': [2048,1024], 'CNT': [2048,1024]}."""
    h_new = (
        np.stack([np.asarray(results[core]["HT"]) for core in range(N_CORES)])
        .transpose(0, 2, 1)
        .reshape(B, DH)
        .astype(np.float32)
    )
    c_new = (
        np.stack([np.asarray(results[core]["CNT"]) for core in range(N_CORES)])
        .transpose(0, 2, 1)
        .reshape(B, DH)
        .astype(np.float32)
    )
    return (h_new, c_new)


def kernel(x, h, c, Wix, bix, Wmx, bmx, Wox, box, Wih, bih, Wmh, bmh, Woh, boh):
    global LAST_RESULT
    from concourse.bass_utils import run_bass_kernel_spmd

    in_maps = _prepare_in_maps(
        x, h, c, Wix, bix, Wmx, bmx, Wox, box, Wih, bih, Wmh, bmh, Woh, boh
    )
    nc = _get_bass()
    try:
        res = run_bass_kernel_spmd(nc, in_maps, core_ids=list(range(N_CORES)))
    except ModuleNotFoundError:
        # BASS_TRACE under axon needs antenv.axon_hooks, which some
        # containers lack; fall back to an untraced run.
        os.environ["BASS_NEVER_TRACE"] = "1"
        res = run_bass_kernel_spmd(nc, in_maps, core_ids=list(range(N_CORES)))
    LAST_RESULT = res
    return _postprocess(res.results)


# revision 26
# speedup vs baseline: 1.5476x; 1.0565x over previous
"""LSTM cell kernel for Trainium2, SPMD over 8 NeuronCores.

Problem: nn_LstmCell — B=8192, D_IN=D_H=2048.
    g = x @ Wx.T + bx + h @ Wh.T + bh          # [B, 3H]
    gi, gm, go = split(g, 3)
    c_new = sigmoid(gm)*c + sigmoid(gi)*tanh(gm)
    h_new = sigmoid(go)*tanh(c_new)

Strategy:
  - Data-parallel over batch: each core owns 1024 rows of x/h/c.
  - Fused GEMM computed transposed (gates on PSUM partitions, batch on the
    free dim) so per-gate biases fold into the ScalarE activation.
  - Mixed precision split by gate sensitivity: the m-gate pre-activation
    feeds tanh (derivative ~1) so it runs in bf16; the i/o gates only feed
    sigmoid (derivative <= 0.25) so they run in fp8 e4m3 with DoubleRow
    perf mode (K=256 per matmul, 2x PE throughput). Measured end-to-end
    rel err ~1.4e-2 vs 2.6e-2 for all-fp8 (gate: 2e-2).
  - fp8 scaling: A*32, W*512 (W's absmax 0.022 is below e4m3's min normal
    0.0156, so unscaled W would quantize to subnormals). The 1/16384
    descale folds into the sigmoid activation's scale operand.
  - DoubleRow matmuls may only write PSUM partitions 0..63, so each i/o
    gate accumulates its two 64-row halves into separate [64, 512] banks;
    the sigmoid ACTs then write the two partition halves of one [128, 512]
    SBUF tile (ScalarE supports input/output partition-base offsets).
  - Weights streamed from HBM (one pass), activations resident in SBUF.

Host-side: layout transforms + bf16/fp8 casts (not counted in HW exec time).
"""

import os

import numpy as np
import ml_dtypes

N_CORES = 8
B = 8192
DH = 2048            # latent dim (= D_IN = D_H)
K = 2 * DH           # 4096 contraction dim
BLOC = B // N_CORES  # 1024 batch rows per core
P = 128
KT = K // P          # 32 k-tiles (128 each)
KG = KT // 2         # 16 doubled k-groups (256 each) for fp8 DoubleRow
DTL = DH // P        # 16 d-tiles per gate
NF = 512             # matmul free dim (one PSUM bank of fp32)
NH = BLOC // NF      # 2 batch halves (vtiles per d-tile)

N_M8 = 3             # leading m-gate d-tiles computed in fp8 (see below)
SA = 32.0            # fp8 activation scale
SW = 512.0           # fp8 weight scale
IO_DESCALE = 1.0 / (SA * SW)

_BF16 = ml_dtypes.bfloat16
_F8 = ml_dtypes.float8_e4m3

_CACHE = {}
LAST_RESULT = None  # BassKernelResults from the most recent run (for test.py)


def _split_multiwaits(nc):
    """This container's walrus build rejects >1 sync-wait on an engine
    instruction ("Too many sync wait commands"). Split extra waits into
    standalone EventSemaphore instructions on the same engine immediately
    before the instruction (same stall semantics: engines are in-order)."""
    import concourse.mybir as mybir

    f = nc.m.functions[0]
    for blk in f.blocks:
        new_insts = []
        for inst in blk.instructions:
            si = getattr(inst, "sync_info", None)
            ow = list(si.on_wait) if (si is not None and si.on_wait) else []
            if len(ow) > 1:
                for w in ow[:-1]:
                    new_insts.append(
                        mybir.InstEventSemaphore(
                            name=nc.get_next_instruction_name(),
                            engine=inst.engine,
                            ins=[],
                            outs=[],
                            sync_info=mybir.SyncInfo(on_wait=[w], on_update=[]),
                        )
                    )
                inst.sync_info = mybir.SyncInfo(
                    on_wait=[ow[-1]], on_update=list(si.on_update)
                )
            new_insts.append(inst)
        blk.instructions[:] = new_insts


def _build_bass(dtl=DTL):
    import concourse.bass as bass
    import concourse.mybir as mybir
    import concourse.tile as tile

    f32 = mybir.dt.float32
    bf16 = mybir.dt.bfloat16
    f8 = mybir.dt.float8e4
    AF = mybir.ActivationFunctionType
    DR = mybir.MatmulPerfMode.DoubleRow

    nc = bass.Bass("TRN2", name="lstm_cell")

    # m-gate weights, bf16: WM[d, p, kt, m] = Wm[d*128+m, kt*128+p]
    WM = nc.dram_tensor("WM", [dtl, P, KT, P], bf16, kind="ExternalInput")
    # i/o gate weights, fp8 DoubleRow layout, plus N_M8 extra strips
    # (index 2*dtl+d) holding the M-GATE's first N_M8 d-tiles in fp8:
    # those tiles run all-fp8, halving their m-matmul time and decoupling
    # the prologue from the 8 MB bf16 A stream (which then loads during
    # the fp8 tiles instead of stalling the pipeline). Each converted tile
    # costs ~1e-3 of rel err (measured: 1 -> 1.51e-2, 3 -> 1.69e-2).
    # W8[g*16+d, p, kg, ii, m] = Wg[d*128+m, kg*256+ii*128+p] * SW
    W8 = nc.dram_tensor("W8", [2 * dtl + N_M8, P, KG, 2, P], f8, kind="ExternalInput")
    # activations A = [x ‖ h], twice: bf16 for m-gate, fp8*SA for i/o
    ABF = nc.dram_tensor("ABF", [P, KT, BLOC], bf16, kind="ExternalInput")
    A8 = nc.dram_tensor("A8", [P, KT, BLOC], f8, kind="ExternalInput")
    CT = nc.dram_tensor("CT", [DH, BLOC], f32, kind="ExternalInput")
    BIAS = nc.dram_tensor("BIAS", [P, 3 * dtl], f32, kind="ExternalInput")
    # i/o biases regrouped per 64-row half (DoubleRow outputs sit at
    # partitions 0..63): BIO[p, ((g*16+d)*2)+b] = bias_g[d*128+b*64+p].
    # Cols 4*dtl + 2*d + b hold the m-gate's bias halves for the N_M8
    # fp8 m-tiles.
    BIO = nc.dram_tensor("BIO", [64, 4 * dtl + 2 * N_M8], f32, kind="ExternalInput")
    HT = nc.dram_tensor("HT", [DH, BLOC], f32, kind="ExternalOutput")
    CNT = nc.dram_tensor("CNT", [DH, BLOC], f32, kind="ExternalOutput")

    with tile.TileContext(nc) as tc:
        with (
            tc.tile_pool(name="const", bufs=1) as const_pool,
            tc.tile_pool(name="wpool", bufs=2) as wpool,
            tc.tile_pool(name="cpool", bufs=2) as cpool,
            tc.tile_pool(name="epool", bufs=3) as epool,
            tc.tile_pool(name="psum", bufs=1, space="PSUM") as psum_pool,
        ):
            # Activations resident in SBUF; per-k-chunk loads so the first
            # d-tile's matmuls can start as soon as early chunks land.
            # Both A streams go on the gpsimd queue, fp8 strictly first:
            # the model serializes all transfers through one shared DMA
            # resource, so only queue order keeps the big bf16 chunks (m-gate,
            # consumed last) from starving the fp8 chunks (i/o, consumed
            # first). Chunks sized to amortize the ~1us SWDGE prep per DMA.
            a8_sb = const_pool.tile([P, KT, BLOC], f8, name="a8_sb")
            for ch in range(8):
                nc.gpsimd.dma_start(
                    a8_sb[:, 4 * ch : 4 * ch + 4, :],
                    A8[:, 4 * ch : 4 * ch + 4, :],
                )
            # ABF chunk DMAs are emitted inside the d==0 body, after d0's
            # wm/c loads, so those beat the bulk bf16 stream to the DMA
            # engines (the m-gate is scheduled last within d0 anyway).
            abf_sb = const_pool.tile([P, KT, BLOC], bf16, name="abf_sb")
            bias_sb = const_pool.tile([P, 3 * dtl], f32, name="bias_sb")
            nc.sync.dma_start(bias_sb[:], BIAS[:])
            bio_sb = const_pool.tile([64, 4 * dtl + 2 * N_M8], f32, name="bio_sb")
            nc.sync.dma_start(bio_sb[:], BIO[:])

            for d in range(dtl):
                # Stream this d-tile's weight strips: fp8 i/o (0.5 MB each)
                # + bf16 m (1 MB).
                # d0's c rides the gpsimd queue between the A8 and ABF
                # streams: it would otherwise win the shared DMA engines
                # ahead of the latency-critical fp8 chunks.
                d0_eng = nc.gpsimd if d == 0 else nc.sync
                w8 = {}
                for gi, g in enumerate("io"):
                    w8[g] = wpool.tile([P, KG, 2, P], f8, name=f"w8{g}", tag=f"w8{g}")
                    nc.sync.dma_start(w8[g][:], W8[gi * dtl + d])
                m_fp8 = d < min(N_M8, dtl)
                if m_fp8:
                    w8["m"] = wpool.tile([P, KG, 2, P], f8, name="w8m", tag="w8m")
                    nc.sync.dma_start(w8["m"][:], W8[2 * dtl + d])
                else:
                    wm = wpool.tile([P, KT, P], bf16, name="wm", tag="wm")
                    nc.sync.dma_start(wm[:], WM[d])

                c_tiles, psums = {}, {}
                for nh in range(NH):
                    c_t = cpool.tile([P, NF], f32, name=f"c_{nh}", tag=f"c_{nh}")
                    d0_eng.dma_start(
                        c_t[:], CT[d * P : (d + 1) * P, nh * NF : (nh + 1) * NF]
                    )
                    c_tiles[nh] = c_t
                    # m-gate: one full bank; parity tags so consecutive
                    # vtiles overlap. i/o: [64, 512] banks (DoubleRow dst
                    # partition must be 0), one per 64-row half. d0's fp8
                    # m-gate gets its own [64, 512] pair (8 banks total).
                    if m_fp8:
                        for b in range(2):
                            psums[("m8", nh, b)] = psum_pool.tile(
                                [64, NF], f32, name=f"ps_m8{b}", tag=f"ps_m8{b}"
                            )
                    else:
                        psums[("m", nh)] = psum_pool.tile(
                            [P, NF], f32, name=f"ps_m{nh}", tag=f"ps_m{nh}"
                        )
                    for g in "io":
                        for b in range(2):
                            psums[(g, nh, b)] = psum_pool.tile(
                                [64, NF], f32, name=f"ps_{g}{b}", tag=f"ps_{g}{b}"
                            )

                def io_matmul(g, nh, b, kg, key=None):
                    # fp8 DoubleRow: K=256 (k-tile pair), M=64, N=512.
                    nc.tensor.matmul(
                        psums[(key or g, nh, b)][:],
                        w8[g][:, kg, :, b * 64 : (b + 1) * 64],
                        a8_sb[:, 2 * kg : 2 * kg + 2, nh * NF : (nh + 1) * NF],
                        start=(kg == 0),
                        stop=(kg == KG - 1),
                        perf_mode=DR,
                    )

                def m_matmul(nh, kt):
                    nc.tensor.matmul(
                        psums[("m", nh)][:],
                        wm[:, kt, :],
                        abf_sb[:, kt, nh * NF : (nh + 1) * NF],
                        start=(kt == 0),
                        stop=(kt == KT - 1),
                    )

                def sig_io(g, gi, s_g, nh, b):
                    nc.scalar.activation(
                        s_g[b * 64 : (b + 1) * 64, :],
                        psums[(g, nh, b)][:],
                        AF.Sigmoid,
                        bias=bio_sb[
                            :, (gi * dtl + d) * 2 + b : (gi * dtl + d) * 2 + b + 1
                        ],
                        scale=IO_DESCALE,
                    )

                def epilogue(nh):
                    # Emission order matters: engines are in-order, so the
                    # o-dependent ops (s_o, h_new) go last — everything else
                    # completes during the o-gate matmuls and only the short
                    # s_o -> h_new chain trails the final matmul.
                    b_m = bias_sb[:, dtl + d : dtl + d + 1]

                    s_i = epool.tile([P, NF], f32, name="s_i", tag="s_i")
                    t_m = epool.tile([P, NF], f32, name="t_m", tag="t_m")
                    s_m = epool.tile([P, NF], f32, name="s_m", tag="s_m")
                    s_o = epool.tile([P, NF], f32, name="s_o", tag="s_o")
                    part = epool.tile([P, NF], f32, name="part", tag="part")
                    fc = epool.tile([P, NF], f32, name="fc", tag="fc")
                    c_new = epool.tile([P, NF], f32, name="c_new", tag="c_new")
                    t_c = epool.tile([P, NF], f32, name="t_c", tag="t_c")
                    h_new = epool.tile([P, NF], f32, name="h_new", tag="h_new")

                    # i halves: PSUM [64, 512] at partition base 0 ->
                    # partition halves of the [128, 512] SBUF tile.
                    for b in range(2):
                        sig_io("i", 0, s_i, nh, b)
                    if m_fp8:
                        for b in range(2):
                            col = 4 * dtl + 2 * d + b
                            bm8 = bio_sb[:, col : col + 1]
                            nc.scalar.activation(
                                t_m[b * 64 : (b + 1) * 64, :],
                                psums[("m8", nh, b)][:],
                                AF.Tanh, bias=bm8, scale=IO_DESCALE,
                            )
                            nc.scalar.activation(
                                s_m[b * 64 : (b + 1) * 64, :],
                                psums[("m8", nh, b)][:],
                                AF.Sigmoid, bias=bm8, scale=IO_DESCALE,
                            )
                    else:
                        nc.scalar.activation(t_m[:], psums[("m", nh)][:], AF.Tanh, bias=b_m)
                        nc.scalar.activation(s_m[:], psums[("m", nh)][:], AF.Sigmoid, bias=b_m)
                    nc.vector.tensor_mul(part[:], s_i[:], t_m[:])
                    nc.vector.tensor_mul(fc[:], s_m[:], c_tiles[nh][:])
                    nc.vector.tensor_add(c_new[:], fc[:], part[:])
                    nc.scalar.activation(t_c[:], c_new[:], AF.Tanh)
                    nc.gpsimd.dma_start(
                        CNT[d * P : (d + 1) * P, nh * NF : (nh + 1) * NF], c_new[:]
                    )
                    for b in range(2):
                        sig_io("o", 1, s_o, nh, b)
                    nc.vector.tensor_mul(h_new[:], s_o[:], t_c[:])
                    nc.gpsimd.dma_start(
                        HT[d * P : (d + 1) * P, nh * NF : (nh + 1) * NF], h_new[:]
                    )

                if d == 0:
                    # Emit the bulk bf16 A stream (needed first by d1's
                    # m-gate) behind d0's c loads on the same queue.
                    for ch in range(8):
                        nc.gpsimd.dma_start(
                            abf_sb[:, 4 * ch : 4 * ch + 4, :],
                            ABF[:, 4 * ch : 4 * ch + 4, :],
                        )
                    # d0 is all-fp8 (m included): kg-major ACROSS gates so
                    # every fp8 A chunk feeds 12 matmuls on arrival and the
                    # PE tracks the DMA stream without long stalls; no
                    # dependence on the bf16 stream at all.
                    for nh in range(NH):
                        for kg in range(KG):
                            for g in "imo":
                                for b in range(2):
                                    io_matmul(g, nh, b, kg, key="m8" if g == "m" else None)
                        epilogue(nh)
                else:
                    # gate-major per vtile, o last: everything except the
                    # short s_o -> h_new chain completes during the o-gate
                    # matmuls (see epilogue()).
                    for nh in range(NH):
                        for b in range(2):
                            for kg in range(KG):
                                io_matmul("i", nh, b, kg)
                        if m_fp8:
                            for b in range(2):
                                for kg in range(KG):
                                    io_matmul("m", nh, b, kg, key="m8")
                        else:
                            for kt in range(KT):
                                m_matmul(nh, kt)
                        for b in range(2):
                            for kg in range(KG):
                                io_matmul("o", nh, b, kg)
                        epilogue(nh)

    _split_multiwaits(nc)
    return nc


def _get_bass():
    if "nc" not in _CACHE:
        _CACHE["nc"] = _build_bass()
    return _CACHE["nc"]


def _prepare_in_maps(x, h, c, Wix, bix, Wmx, bmx, Wox, box, Wih, bih, Wmh, bmh, Woh, boh):
    x = np.asarray(x, dtype=np.float32)
    h = np.asarray(h, dtype=np.float32)
    c = np.asarray(c, dtype=np.float32)

    # Per-gate fused weights [2048, 4096]: W = [Wx ‖ Wh]
    Wg = {
        "i": np.concatenate([np.asarray(Wix), np.asarray(Wih)], axis=1),
        "m": np.concatenate([np.asarray(Wmx), np.asarray(Wmh)], axis=1),
        "o": np.concatenate([np.asarray(Wox), np.asarray(Woh)], axis=1),
    }

    # m-gate bf16: WM[d, p, kt, m] = Wm[d*128+m, kt*128+p]
    WM_host = np.ascontiguousarray(
        Wg["m"].astype(np.float32).reshape(DTL, P, KT, P).transpose(0, 3, 2, 1)
    ).astype(_BF16)

    # i/o gates fp8 (scaled by SW), DoubleRow layout:
    # W8[g*16+d, p, kg, ii, m] = Wg[d*128+m, kg*256+ii*128+p]*SW
    # plus the m-gate's d=0 strip at index 2*DTL (d-tile 0 runs all-fp8).
    w8_list = []
    for g in "io":
        ws = (Wg[g].astype(np.float32) * SW).astype(_F8)
        w8_list.append(ws.reshape(DTL, P, KG, 2, P).transpose(0, 4, 2, 3, 1))
    wm8 = (Wg["m"][: N_M8 * P].astype(np.float32) * SW).astype(_F8)
    w8_list.append(wm8.reshape(N_M8, P, KG, 2, P).transpose(0, 4, 2, 3, 1))
    W8_host = np.ascontiguousarray(np.concatenate(w8_list, axis=0))

    # A = [x ‖ h] : [8192, 4096] -> per-core [p, kt, n], in bf16 and fp8*SA
    A = np.concatenate([x, h], axis=1)
    A_t = A.reshape(N_CORES, BLOC, KT, P).transpose(0, 3, 2, 1)
    ABF_host = np.ascontiguousarray(A_t).astype(_BF16)
    A8_host = np.ascontiguousarray(A_t * np.float32(SA)).astype(_F8)

    # c transposed per core: [core, 2048, 1024]
    CT_host = np.ascontiguousarray(c.reshape(N_CORES, BLOC, DH).transpose(0, 2, 1))

    bias = {g: (np.asarray(bx) + np.asarray(bh)).astype(np.float32)
            for g, bx, bh in (("i", bix, bih), ("m", bmx, bmh), ("o", box, boh))}
    BIAS_host = np.ascontiguousarray(
        np.concatenate([bias["i"], bias["m"], bias["o"]]).reshape(3 * DTL, P).T
    )
    # BIO[p, (g*16+d)*2+b] = bias_g[d*128+b*64+p] for g in (i, o);
    # trailing 2*N_M8 cols: m-gate bias halves for the fp8 m-tiles.
    BIO_host = np.ascontiguousarray(
        np.concatenate([bias["i"], bias["o"], bias["m"][: N_M8 * P]])
        .reshape(4 * DTL + 2 * N_M8, 64)
        .T
    )

    return [
        {
            "WM": WM_host,
            "W8": W8_host,
            "ABF": ABF_host[core],
            "A8": A8_host[core],
            "CT": CT_host[core],
            "BIAS": BIAS_host,
            "BIO": BIO_host,
        }
        for core in range(N_CORES)
    ]


def _postprocess(results):
    """results: per-core list of {'HT': [2048,1024], 'CNT': [2048,1024]}."""
    h_new = (
        np.stack([np.asarray(results[core]["HT"]) for core in range(N_CORES)])
        .transpose(0, 2, 1)
        .reshape(B, DH)
        .astype(np.float32)
    )
    c_new = (
        np.stack([np.asarray(results[core]["CNT"]) for core in range(N_CORES)])
        .transpose(0, 2, 1)
        .reshape(B, DH)
        .astype(np.float32)
    )
    return (h_new, c_new)


def kernel(x, h, c, Wix, bix, Wmx, bmx, Wox, box, Wih, bih, Wmh, bmh, Woh, boh):
    global LAST_RESULT
    from concourse.bass_utils import run_bass_kernel_spmd

    in_maps = _prepare_in_maps(
        x, h, c, Wix, bix, Wmx, bmx, Wox, box, Wih, bih, Wmh, bmh, Woh, boh
    )
    nc = _get_bass()
    try:
        res = run_bass_kernel_spmd(nc, in_maps, core_ids=list(range(N_CORES)))
    except ModuleNotFoundError:
        # BASS_TRACE under axon needs antenv.axon_hooks, which some
        # containers lack; fall back to an untraced run.
        os.environ["BASS_NEVER_TRACE"] = "1"
        res = run_bass_kernel_spmd(nc, in_maps, core_ids=list(range(N_CORES)))
    LAST_RESULT = res
    return _postprocess(res.results)


# revision 29
# speedup vs baseline: 1.5525x; 1.0032x over previous
"""LSTM cell kernel for Trainium2, SPMD over 8 NeuronCores.

Problem: nn_LstmCell — B=8192, D_IN=D_H=2048.
    g = x @ Wx.T + bx + h @ Wh.T + bh          # [B, 3H]
    gi, gm, go = split(g, 3)
    c_new = sigmoid(gm)*c + sigmoid(gi)*tanh(gm)
    h_new = sigmoid(go)*tanh(c_new)

Strategy:
  - Data-parallel over batch: each core owns 1024 rows of x/h/c.
  - Fused GEMM computed transposed (gates on PSUM partitions, batch on the
    free dim) so per-gate biases fold into the ScalarE activation.
  - Mixed precision split by gate sensitivity: the m-gate pre-activation
    feeds tanh (derivative ~1) so it runs in bf16; the i/o gates only feed
    sigmoid (derivative <= 0.25) so they run in fp8 e4m3 with DoubleRow
    perf mode (K=256 per matmul, 2x PE throughput). Measured end-to-end
    rel err ~1.4e-2 vs 2.6e-2 for all-fp8 (gate: 2e-2).
  - fp8 scaling: A*32, W*512 (W's absmax 0.022 is below e4m3's min normal
    0.0156, so unscaled W would quantize to subnormals). The 1/16384
    descale folds into the sigmoid activation's scale operand.
  - DoubleRow matmuls may only write PSUM partitions 0..63, so each i/o
    gate accumulates its two 64-row halves into separate [64, 512] banks;
    the sigmoid ACTs then write the two partition halves of one [128, 512]
    SBUF tile (ScalarE supports input/output partition-base offsets).
  - Weights streamed from HBM (one pass), activations resident in SBUF.

Host-side: layout transforms + bf16/fp8 casts (not counted in HW exec time).
"""

import os

import numpy as np
import ml_dtypes

N_CORES = 8
B = 8192
DH = 2048            # latent dim (= D_IN = D_H)
K = 2 * DH           # 4096 contraction dim
BLOC = B // N_CORES  # 1024 batch rows per core
P = 128
KT = K // P          # 32 k-tiles (128 each)
KG = KT // 2         # 16 doubled k-groups (256 each) for fp8 DoubleRow
DTL = DH // P        # 16 d-tiles per gate
NF = 512             # matmul free dim (one PSUM bank of fp32)
NH = BLOC // NF      # 2 batch halves (vtiles per d-tile)

N_M8 = 3             # leading m-gate d-tiles computed in fp8 (see below)
SA = 32.0            # fp8 activation scale
SW = 512.0           # fp8 weight scale
IO_DESCALE = 1.0 / (SA * SW)

_BF16 = ml_dtypes.bfloat16
_F8 = ml_dtypes.float8_e4m3

_CACHE = {}
LAST_RESULT = None  # BassKernelResults from the most recent run (for test.py)


def _split_multiwaits(nc):
    """This container's walrus build rejects >1 sync-wait on an engine
    instruction ("Too many sync wait commands"). Split extra waits into
    standalone EventSemaphore instructions on the same engine immediately
    before the instruction (same stall semantics: engines are in-order)."""
    import concourse.mybir as mybir

    f = nc.m.functions[0]
    for blk in f.blocks:
        new_insts = []
        for inst in blk.instructions:
            si = getattr(inst, "sync_info", None)
            ow = list(si.on_wait) if (si is not None and si.on_wait) else []
            if len(ow) > 1:
                for w in ow[:-1]:
                    new_insts.append(
                        mybir.InstEventSemaphore(
                            name=nc.get_next_instruction_name(),
                            engine=inst.engine,
                            ins=[],
                            outs=[],
                            sync_info=mybir.SyncInfo(on_wait=[w], on_update=[]),
                        )
                    )
                inst.sync_info = mybir.SyncInfo(
                    on_wait=[ow[-1]], on_update=list(si.on_update)
                )
            new_insts.append(inst)
        blk.instructions[:] = new_insts


def _build_bass(dtl=DTL):
    import concourse.bass as bass
    import concourse.mybir as mybir
    import concourse.tile as tile

    f32 = mybir.dt.float32
    bf16 = mybir.dt.bfloat16
    f8 = mybir.dt.float8e4
    AF = mybir.ActivationFunctionType
    DR = mybir.MatmulPerfMode.DoubleRow

    nc = bass.Bass("TRN2", name="lstm_cell")

    # m-gate weights, bf16: WM[d, p, kt, m] = Wm[d*128+m, kt*128+p]
    WM = nc.dram_tensor("WM", [dtl, P, KT, P], bf16, kind="ExternalInput")
    # i/o gate weights, fp8 DoubleRow layout, plus N_M8 extra strips
    # (index 2*dtl+d) holding the M-GATE's first N_M8 d-tiles in fp8:
    # those tiles run all-fp8, halving their m-matmul time and decoupling
    # the prologue from the 8 MB bf16 A stream (which then loads during
    # the fp8 tiles instead of stalling the pipeline). Each converted tile
    # costs ~1e-3 of rel err (measured: 1 -> 1.51e-2, 3 -> 1.69e-2).
    # W8[g*16+d, p, kg, ii, m] = Wg[d*128+m, kg*256+ii*128+p] * SW
    W8 = nc.dram_tensor("W8", [2 * dtl + N_M8, P, KG, 2, P], f8, kind="ExternalInput")
    # activations A = [x ‖ h], twice: bf16 for m-gate, fp8*SA for i/o
    ABF = nc.dram_tensor("ABF", [P, KT, BLOC], bf16, kind="ExternalInput")
    A8 = nc.dram_tensor("A8", [P, KT, BLOC], f8, kind="ExternalInput")
    CT = nc.dram_tensor("CT", [DH, BLOC], f32, kind="ExternalInput")
    BIAS = nc.dram_tensor("BIAS", [P, 3 * dtl], f32, kind="ExternalInput")
    # i/o biases regrouped per 64-row half (DoubleRow outputs sit at
    # partitions 0..63): BIO[p, ((g*16+d)*2)+b] = bias_g[d*128+b*64+p].
    # Cols 4*dtl + 2*d + b hold the m-gate's bias halves for the N_M8
    # fp8 m-tiles.
    BIO = nc.dram_tensor("BIO", [64, 4 * dtl + 2 * N_M8], f32, kind="ExternalInput")
    HT = nc.dram_tensor("HT", [DH, BLOC], f32, kind="ExternalOutput")
    CNT = nc.dram_tensor("CNT", [DH, BLOC], f32, kind="ExternalOutput")

    with tile.TileContext(nc) as tc:
        with (
            tc.tile_pool(name="const", bufs=1) as const_pool,
            tc.tile_pool(name="wpool", bufs=2) as wpool,
            tc.tile_pool(name="cpool", bufs=2) as cpool,
            tc.tile_pool(name="epool", bufs=3) as epool,
            tc.tile_pool(name="psum", bufs=1, space="PSUM") as psum_pool,
        ):
            # Activations resident in SBUF; per-k-chunk loads so the first
            # d-tile's matmuls can start as soon as early chunks land.
            # Both A streams go on the gpsimd queue, fp8 strictly first:
            # the model serializes all transfers through one shared DMA
            # resource, so only queue order keeps the big bf16 chunks (m-gate,
            # consumed last) from starving the fp8 chunks (i/o, consumed
            # first). Chunks sized to amortize the ~1us SWDGE prep per DMA.
            a8_sb = const_pool.tile([P, KT, BLOC], f8, name="a8_sb")
            for ch in range(8):
                nc.gpsimd.dma_start(
                    a8_sb[:, 4 * ch : 4 * ch + 4, :],
                    A8[:, 4 * ch : 4 * ch + 4, :],
                )
            # ABF chunk DMAs are emitted inside the d==0 body, after d0's
            # wm/c loads, so those beat the bulk bf16 stream to the DMA
            # engines (the m-gate is scheduled last within d0 anyway).
            abf_sb = const_pool.tile([P, KT, BLOC], bf16, name="abf_sb")
            bias_sb = const_pool.tile([P, 3 * dtl], f32, name="bias_sb")
            nc.sync.dma_start(bias_sb[:], BIAS[:])
            bio_sb = const_pool.tile([64, 4 * dtl + 2 * N_M8], f32, name="bio_sb")
            nc.sync.dma_start(bio_sb[:], BIO[:])

            for d in range(dtl):
                # Stream this d-tile's weight strips: fp8 i/o (0.5 MB each)
                # + bf16 m (1 MB).
                # d0's c rides the gpsimd queue between the A8 and ABF
                # streams: it would otherwise win the shared DMA engines
                # ahead of the latency-critical fp8 chunks.
                d0_eng = nc.gpsimd if d == 0 else nc.sync
                m_fp8 = d < min(N_M8, dtl)
                gates = [("i", d), ("o", dtl + d)]
                if m_fp8:
                    # i, m, o order matches d0's kg-major gate order
                    gates.insert(1 if d == 0 else 2, ("m", 2 * dtl + d))
                w8 = {}
                for g, idx in gates:
                    w8[g] = wpool.tile([P, KG, 2, P], f8, name=f"w8{g}", tag=f"w8{g}")
                    if d == 0:
                        # halves so the first kg's weights land sooner
                        for hf in range(2):
                            nc.sync.dma_start(
                                w8[g][:, 8 * hf : 8 * hf + 8],
                                W8[idx][:, 8 * hf : 8 * hf + 8],
                            )
                    else:
                        nc.sync.dma_start(w8[g][:], W8[idx])
                if not m_fp8:
                    wm = wpool.tile([P, KT, P], bf16, name="wm", tag="wm")
                    nc.sync.dma_start(wm[:], WM[d])

                c_tiles, psums = {}, {}
                for nh in range(NH):
                    c_t = cpool.tile([P, NF], f32, name=f"c_{nh}", tag=f"c_{nh}")
                    d0_eng.dma_start(
                        c_t[:], CT[d * P : (d + 1) * P, nh * NF : (nh + 1) * NF]
                    )
                    c_tiles[nh] = c_t
                    # m-gate: one full bank; parity tags so consecutive
                    # vtiles overlap. i/o: [64, 512] banks (DoubleRow dst
                    # partition must be 0), one per 64-row half. d0's fp8
                    # m-gate gets its own [64, 512] pair (8 banks total).
                    if m_fp8:
                        for b in range(2):
                            psums[("m8", nh, b)] = psum_pool.tile(
                                [64, NF], f32, name=f"ps_m8{b}", tag=f"ps_m8{b}"
                            )
                    else:
                        psums[("m", nh)] = psum_pool.tile(
                            [P, NF], f32, name=f"ps_m{nh}", tag=f"ps_m{nh}"
                        )
                    for g in "io":
                        for b in range(2):
                            psums[(g, nh, b)] = psum_pool.tile(
                                [64, NF], f32, name=f"ps_{g}{b}", tag=f"ps_{g}{b}"
                            )

                def io_matmul(g, nh, b, kg, key=None):
                    # fp8 DoubleRow: K=256 (k-tile pair), M=64, N=512.
                    nc.tensor.matmul(
                        psums[(key or g, nh, b)][:],
                        w8[g][:, kg, :, b * 64 : (b + 1) * 64],
                        a8_sb[:, 2 * kg : 2 * kg + 2, nh * NF : (nh + 1) * NF],
                        start=(kg == 0),
                        stop=(kg == KG - 1),
                        perf_mode=DR,
                    )

                def m_matmul(nh, kt):
                    nc.tensor.matmul(
                        psums[("m", nh)][:],
                        wm[:, kt, :],
                        abf_sb[:, kt, nh * NF : (nh + 1) * NF],
                        start=(kt == 0),
                        stop=(kt == KT - 1),
                    )

                def sig_io(g, gi, s_g, nh, b):
                    nc.scalar.activation(
                        s_g[b * 64 : (b + 1) * 64, :],
                        psums[(g, nh, b)][:],
                        AF.Sigmoid,
                        bias=bio_sb[
                            :, (gi * dtl + d) * 2 + b : (gi * dtl + d) * 2 + b + 1
                        ],
                        scale=IO_DESCALE,
                    )

                def epilogue(nh):
                    # Emission order matters: engines are in-order, so the
                    # o-dependent ops (s_o, h_new) go last — everything else
                    # completes during the o-gate matmuls and only the short
                    # s_o -> h_new chain trails the final matmul.
                    b_m = bias_sb[:, dtl + d : dtl + d + 1]

                    s_i = epool.tile([P, NF], f32, name="s_i", tag="s_i")
                    t_m = epool.tile([P, NF], f32, name="t_m", tag="t_m")
                    s_m = epool.tile([P, NF], f32, name="s_m", tag="s_m")
                    s_o = epool.tile([P, NF], f32, name="s_o", tag="s_o")
                    part = epool.tile([P, NF], f32, name="part", tag="part")
                    fc = epool.tile([P, NF], f32, name="fc", tag="fc")
                    c_new = epool.tile([P, NF], f32, name="c_new", tag="c_new")
                    t_c = epool.tile([P, NF], f32, name="t_c", tag="t_c")
                    h_new = epool.tile([P, NF], f32, name="h_new", tag="h_new")

                    # i halves: PSUM [64, 512] at partition base 0 ->
                    # partition halves of the [128, 512] SBUF tile.
                    for b in range(2):
                        sig_io("i", 0, s_i, nh, b)
                    if m_fp8:
                        for b in range(2):
                            col = 4 * dtl + 2 * d + b
                            bm8 = bio_sb[:, col : col + 1]
                            nc.scalar.activation(
                                t_m[b * 64 : (b + 1) * 64, :],
                                psums[("m8", nh, b)][:],
                                AF.Tanh, bias=bm8, scale=IO_DESCALE,
                            )
                            nc.scalar.activation(
                                s_m[b * 64 : (b + 1) * 64, :],
                                psums[("m8", nh, b)][:],
                                AF.Sigmoid, bias=bm8, scale=IO_DESCALE,
                            )
                    else:
                        nc.scalar.activation(t_m[:], psums[("m", nh)][:], AF.Tanh, bias=b_m)
                        nc.scalar.activation(s_m[:], psums[("m", nh)][:], AF.Sigmoid, bias=b_m)
                    nc.vector.tensor_mul(part[:], s_i[:], t_m[:])
                    nc.vector.tensor_mul(fc[:], s_m[:], c_tiles[nh][:])
                    nc.vector.tensor_add(c_new[:], fc[:], part[:])
                    nc.scalar.activation(t_c[:], c_new[:], AF.Tanh)
                    nc.sync.dma_start(
                        CNT[d * P : (d + 1) * P, nh * NF : (nh + 1) * NF], c_new[:]
                    )
                    if d == dtl - 1 and nh == NH - 1:
                        # Final vtile: halve the o-dependent chain so the
                        # first h_new DMA overlaps the second half's compute.
                        for hf in range(2):
                            cs = hf * (NF // 2)
                            for b in range(2):
                                col = (dtl + d) * 2 + b
                                nc.scalar.activation(
                                    s_o[b * 64 : (b + 1) * 64, cs : cs + NF // 2],
                                    psums[("o", nh, b)][:, cs : cs + NF // 2],
                                    AF.Sigmoid,
                                    bias=bio_sb[:, col : col + 1],
                                    scale=IO_DESCALE,
                                )
                            nc.vector.tensor_mul(
                                h_new[:, cs : cs + NF // 2],
                                s_o[:, cs : cs + NF // 2],
                                t_c[:, cs : cs + NF // 2],
                            )
                            nc.sync.dma_start(
                                HT[d * P : (d + 1) * P,
                                   nh * NF + cs : nh * NF + cs + NF // 2],
                                h_new[:, cs : cs + NF // 2],
                            )
                    else:
                        for b in range(2):
                            sig_io("o", 1, s_o, nh, b)
                        nc.vector.tensor_mul(h_new[:], s_o[:], t_c[:])
                        nc.sync.dma_start(
                            HT[d * P : (d + 1) * P, nh * NF : (nh + 1) * NF], h_new[:]
                        )

                if d == 0:
                    # Emit the bulk bf16 A stream (needed first by d1's
                    # m-gate) behind d0's c loads on the same queue.
                    for ch in range(8):
                        nc.gpsimd.dma_start(
                            abf_sb[:, 4 * ch : 4 * ch + 4, :],
                            ABF[:, 4 * ch : 4 * ch + 4, :],
                        )
                    # d0 is all-fp8 (m included): kg-major ACROSS gates so
                    # every fp8 A chunk feeds 12 matmuls on arrival and the
                    # PE tracks the DMA stream without long stalls; no
                    # dependence on the bf16 stream at all.
                    for nh in range(NH):
                        for kg in range(KG):
                            for g in "imo":
                                for b in range(2):
                                    io_matmul(g, nh, b, kg, key="m8" if g == "m" else None)
                        epilogue(nh)
                else:
                    # gate-major per vtile, o last: everything except the
                    # short s_o -> h_new chain completes during the o-gate
                    # matmuls (see epilogue()).
                    for nh in range(NH):
                        for b in range(2):
                            for kg in range(KG):
                                io_matmul("i", nh, b, kg)
                        if m_fp8:
                            for b in range(2):
                                for kg in range(KG):
                                    io_matmul("m", nh, b, kg, key="m8")
                        else:
                            for kt in range(KT):
                                m_matmul(nh, kt)
                        for b in range(2):
                            for kg in range(KG):
                                io_matmul("o", nh, b, kg)
                        epilogue(nh)

    _split_multiwaits(nc)
    return nc


def _get_bass():
    if "nc" not in _CACHE:
        _CACHE["nc"] = _build_bass()
    return _CACHE["nc"]


def _prepare_in_maps(x, h, c, Wix, bix, Wmx, bmx, Wox, box, Wih, bih, Wmh, bmh, Woh, boh):
    x = np.asarray(x, dtype=np.float32)
    h = np.asarray(h, dtype=np.float32)
    c = np.asarray(c, dtype=np.float32)

    # Per-gate fused weights [2048, 4096]: W = [Wx ‖ Wh]
    Wg = {
        "i": np.concatenate([np.asarray(Wix), np.asarray(Wih)], axis=1),
        "m": np.concatenate([np.asarray(Wmx), np.asarray(Wmh)], axis=1),
        "o": np.concatenate([np.asarray(Wox), np.asarray(Woh)], axis=1),
    }

    # m-gate bf16: WM[d, p, kt, m] = Wm[d*128+m, kt*128+p]
    WM_host = np.ascontiguousarray(
        Wg["m"].astype(np.float32).reshape(DTL, P, KT, P).transpose(0, 3, 2, 1)
    ).astype(_BF16)

    # i/o gates fp8 (scaled by SW), DoubleRow layout:
    # W8[g*16+d, p, kg, ii, m] = Wg[d*128+m, kg*256+ii*128+p]*SW
    # plus the m-gate's d=0 strip at index 2*DTL (d-tile 0 runs all-fp8).
    w8_list = []
    for g in "io":
        ws = (Wg[g].astype(np.float32) * SW).astype(_F8)
        w8_list.append(ws.reshape(DTL, P, KG, 2, P).transpose(0, 4, 2, 3, 1))
    wm8 = (Wg["m"][: N_M8 * P].astype(np.float32) * SW).astype(_F8)
    w8_list.append(wm8.reshape(N_M8, P, KG, 2, P).transpose(0, 4, 2, 3, 1))
    W8_host = np.ascontiguousarray(np.concatenate(w8_list, axis=0))

    # A = [x ‖ h] : [8192, 4096] -> per-core [p, kt, n], in bf16 and fp8*SA
    A = np.concatenate([x, h], axis=1)
    A_t = A.reshape(N_CORES, BLOC, KT, P).transpose(0, 3, 2, 1)
    ABF_host = np.ascontiguousarray(A_t).astype(_BF16)
    A8_host = np.ascontiguousarray(A_t * np.float32(SA)).astype(_F8)

    # c transposed per core: [core, 2048, 1024]
    CT_host = np.ascontiguousarray(c.reshape(N_CORES, BLOC, DH).transpose(0, 2, 1))

    bias = {g: (np.asarray(bx) + np.asarray(bh)).astype(np.float32)
            for g, bx, bh in (("i", bix, bih), ("m", bmx, bmh), ("o", box, boh))}
    BIAS_host = np.ascontiguousarray(
        np.concatenate([bias["i"], bias["m"], bias["o"]]).reshape(3 * DTL, P).T
    )
    # BIO[p, (g*16+d)*2+b] = bias_g[d*128+b*64+p] for g in (i, o);
    # trailing 2*N_M8 cols: m-gate bias halves for the fp8 m-tiles.
    BIO_host = np.ascontiguousarray(
        np.concatenate([bias["i"], bias["o"], bias["m"][: N_M8 * P]])
        .reshape(4 * DTL + 2 * N_M8, 64)
        .T
    )

    return [
        {
            "WM": WM_host,
            "W8": W8_host,
            "ABF": ABF_host[core],
            "A8": A8_host[core],
            "CT": CT_host[core],
            "BIAS": BIAS_host,
            "BIO": BIO_host,
        }
        for core in range(N_CORES)
    ]


def _postprocess(results):
    """results: per-core list of {'HT': [2048,1024], 'CNT': [2048,1024]}."""
    h_new = (
        np.stack([np.asarray(results[core]["HT"]) for core in range(N_CORES)])
        .transpose(0, 2, 1)
        .reshape(B, DH)
        .astype(np.float32)
    )
    c_new = (
        np.stack([np.asarray(results[core]["CNT"]) for core in range(N_CORES)])
        .transpose(0, 2, 1)
        .reshape(B, DH)
        .astype(np.float32)
    )
    return (h_new, c_new)


def kernel(x, h, c, Wix, bix, Wmx, bmx, Wox, box, Wih, bih, Wmh, bmh, Woh, boh):
    global LAST_RESULT
    from concourse.bass_utils import run_bass_kernel_spmd

    in_maps = _prepare_in_maps(
        x, h, c, Wix, bix, Wmx, bmx, Wox, box, Wih, bih, Wmh, bmh, Woh, boh
    )
    nc = _get_bass()
    try:
        res = run_bass_kernel_spmd(nc, in_maps, core_ids=list(range(N_CORES)))
    except ModuleNotFoundError:
        # BASS_TRACE under axon needs antenv.axon_hooks, which some
        # containers lack; fall back to an untraced run.
        os.environ["BASS_NEVER_TRACE"] = "1"
        res = run_bass_kernel_spmd(nc, in_maps, core_ids=list(range(N_CORES)))
    LAST_RESULT = res
    return _postprocess(res.results)
